# revision 4
# baseline (speedup 1.0000x reference)
"""MLA (MultiHeadLatentAttention) Trainium2 kernel, 8-core SPMD.

Sharding: batch x sequence. Core c handles batch c//4, seq rows
[(c%4)*512, (c%4+1)*512). Projections are computed locally on each core's
512-row chunk; k2/v are AllGathered within each 4-core batch group for
attention; attention q-rows stay local.

Self-contained: hardcodes all shapes, no file reads.
"""
import sys

sys.path.insert(0, "/opt/trn_rl_repo")

import numpy as np

B, S, E, H, HD = 2, 2048, 1024, 16, 64
C, L = 512, 512
P = 128
SC = 512          # seq rows per core
NCORES = 8
NT = E // P       # 8 feature tiles
RG = [[0, 1, 2, 3], [4, 5, 6, 7]]

_cache = {}


def _build_nc():
    import concourse.mybir as mybir
    import concourse.tile as tile
    from concourse import bacc

    f16 = mybir.dt.float16
    f32 = mybir.dt.float32
    AF = mybir.ActivationFunctionType

    nc = bacc.Bacc("TRN2", target_bir_lowering=False, debug=False,
                   num_devices=NCORES)

    def din(name, shape, dt=f16):
        return nc.dram_tensor(name, shape, dt, kind="ExternalInput").ap()

    qryT = din("qryT", [P, NT, SC])
    keyT = din("keyT", [P, NT, SC])
    valT = din("valT", [P, NT, SC])
    wq = din("wq", [P, NT, E])
    wqr = din("wqr", [P, NT, E])
    wk = din("wk", [P, NT, E])
    wkr = din("wkr", [P, NT, E])
    wv = din("wv", [P, NT, E])
    wcv = din("wcv", [P, NT, L])
    wdk = din("wdk", [P, 16, L])
    wo = din("wo", [P, NT, E])
    bq = din("bq", [P, NT], f32)
    bqr = din("bqr", [P, NT], f32)
    bk = din("bk", [P, NT], f32)
    bkr = din("bkr", [P, NT], f32)
    bv = din("bv", [P, E], f32)
    bcv = din("bcv", [P, L], f32)
    bdk = din("bdk", [P, L], f32)
    bo = din("bo", [P, E], f32)
    cos0 = din("cos0", [P, SC])
    sin0 = din("sin0", [P, SC])
    perm = din("perm", [P, P])

    w_out = nc.dram_tensor("w_out", [H, SC, S], f16, kind="ExternalOutput").ap()
    o_out = nc.dram_tensor("o_out", [SC, E], f32, kind="ExternalOutput").ap()
    ck_out = nc.dram_tensor("ck_out", [SC, L], f32, kind="ExternalOutput").ap()
    cv_out = nc.dram_tensor("cv_out", [SC, L], f32, kind="ExternalOutput").ap()

    with tile.TileContext(nc) as tc:
        with (
            tc.tile_pool(name="consts", bufs=1) as consts,
            tc.tile_pool(name="inbuf", bufs=1) as inbuf,
            tc.tile_pool(name="wpool", bufs=2) as wpool,
            tc.tile_pool(name="acts", bufs=1) as acts,
            tc.tile_pool(name="str3", bufs=3) as str3,
            tc.tile_pool(name="k2hp", bufs=2) as k2hp,
            tc.tile_pool(name="expp", bufs=1) as expp,
            tc.tile_pool(name="zp", bufs=8) as zp,
            tc.tile_pool(name="psProj", bufs=2, space="PSUM") as psProj,
            tc.tile_pool(name="psB", bufs=2, space="PSUM") as psB,
            tc.tile_pool(name="psAV", bufs=2, space="PSUM") as psAV,
            tc.tile_pool(name="dramp", bufs=1, space="DRAM") as dramp,
        ):
            # ---- constants / biases ----
            def load(pool, ap, shape, dt, name):
                t = pool.tile(shape, dt, name=name)
                nc.sync.dma_start(t[:], ap)
                return t

            cos0_sb = load(consts, cos0, [P, SC], f16, "cos0_sb")
            sin0_sb = load(consts, sin0, [P, SC], f16, "sin0_sb")
            perm_sb = load(consts, perm, [P, P], f16, "perm_sb")
            bq_sb = load(consts, bq, [P, NT], f32, "bq_sb")
            bqr_sb = load(consts, bqr, [P, NT], f32, "bqr_sb")
            bk_sb = load(consts, bk, [P, NT], f32, "bk_sb")
            bkr_sb = load(consts, bkr, [P, NT], f32, "bkr_sb")
            bv_sb = load(consts, bv, [P, E], f32, "bv_sb")
            bcv_sb = load(consts, bcv, [P, L], f32, "bcv_sb")
            bdk_sb = load(consts, bdk, [P, L], f32, "bdk_sb")
            bo_sb = load(consts, bo, [P, E], f32, "bo_sb")

            def load_xT(ap, name):
                t = inbuf.tile([P, NT, SC], f16, name=name, tag="xT", bufs=2)
                nc.sync.dma_start(t[:], ap)
                return t

            keyT_sb = load_xT(keyT, "keyT_sb")
            valT_sb = load_xT(valT, "valT_sb")

            # persistent activations
            k2T_sb = acts.tile([P, 16, SC], f16, name="k2T_sb", tag="kq2T")
            v_sb = acts.tile([P, 4, E], f16, name="v_sb", tag="m8k")
            v2_sb = acts.tile([P, 16, E], f16, name="v2_sb")
            outT64 = acts.tile([64, H, SC], f16, name="outT64")

            def projT(dst_sb, dst_base, w_ap, w_name, x_sb, bias_sb, rope):
                """Transposed-layout linear: dst[:, dst_base+ot, :] =
                f16(W.T-block @ x + bias), with rope rotation on tile 0."""
                w_sb = wpool.tile([P, NT, E], f16, name="w_sb", tag="wbig")
                nc.sync.dma_start(w_sb[:], w_ap)
                for ot in range(NT):
                    ps = psProj.tile([P, SC], f32, name="ps_proj")
                    for it in range(NT):
                        nc.tensor.matmul(
                            ps[:], w_sb[:, it, ot * P:(ot + 1) * P],
                            x_sb[:, it, :],
                            start=(it == 0), stop=(it == NT - 1),
                        )
                    dst = dst_sb[:, dst_base + ot, :]
                    if rope and ot == 0:
                        tmp = str3.tile([P, SC], f16, name="rope_tmp")
                        nc.vector.tensor_scalar_add(tmp[:], ps[:],
                                                    bias_sb[:, 0:1])
                        ps2 = psProj.tile([P, SC], f32, name="ps_proj")
                        nc.tensor.matmul(ps2[:], perm_sb[:], tmp[:],
                                         start=True, stop=True)
                        sw = str3.tile([P, SC], f16, name="rope_sw")
                        nc.scalar.activation(sw[:], ps2[:], AF.Copy)
                        nc.vector.tensor_mul(dst, tmp[:], cos0_sb[:])
                        tmp2 = str3.tile([P, SC], f16, name="rope_tmp2")
                        nc.vector.tensor_mul(tmp2[:], sw[:], sin0_sb[:])
                        nc.vector.tensor_add(dst, dst, tmp2[:])
                    else:
                        nc.vector.tensor_scalar_add(dst, ps[:],
                                                    bias_sb[:, ot:ot + 1])

            # ---- phase K: k2T = [k ; rope(kr_lin)] ----
            projT(k2T_sb, 0, wk, "wk", keyT_sb, bk_sb, rope=False)
            projT(k2T_sb, NT, wkr, "wkr", keyT_sb, bkr_sb, rope=True)

            cc_k_in = dramp.tile([S, SC], f16, name="cc_k_in")
            cc_k_out = dramp.tile([4, S, SC], f16, name="cc_k_out")
            nc.sync.dma_start(
                cc_k_in[:].rearrange("(o p) s -> p o s", p=P), k2T_sb[:])
            nc.gpsimd.collective_compute(
                "AllGather", mybir.AluOpType.bypass, replica_groups=RG,
                ins=[cc_k_in[:].opt()], outs=[cc_k_out[:].opt()])

            # ---- v natural layout [s, e] ----
            wv_sb = wpool.tile([P, NT, E], f16, name="w_sb", tag="wbig")
            nc.sync.dma_start(wv_sb[:], wv)
            for st in range(4):
                for oc in range(2):
                    ps = psProj.tile([P, SC], f32, name="ps_proj")
                    for it in range(NT):
                        nc.tensor.matmul(
                            ps[:], valT_sb[:, it, st * P:(st + 1) * P],
                            wv_sb[:, it, oc * 512:(oc + 1) * 512],
                            start=(it == 0), stop=(it == NT - 1),
                        )
                    nc.vector.tensor_add(
                        v_sb[:, st, oc * 512:(oc + 1) * 512], ps[:],
                        bv_sb[:, oc * 512:(oc + 1) * 512])

            cc_v_in = dramp.tile([SC, E], f16, name="cc_v_in")
            cc_v_out = dramp.tile([4, SC, E], f16, name="cc_v_out")
            nc.sync.dma_start(
                cc_v_in[:].rearrange("(st p) e -> p st e", p=P), v_sb[:])
            nc.gpsimd.collective_compute(
                "AllGather", mybir.AluOpType.bypass, replica_groups=RG,
                ins=[cc_v_in[:].opt()], outs=[cc_v_out[:].opt()])

            # ---- cache_v = value @ Wcv.T + bcv (folded W_DKV @ W_V) ----
            wcv_sb = wpool.tile([P, NT, L], f16, name="wcv_sb", tag="wbig")
            nc.sync.dma_start(wcv_sb[:], wcv)
            for st in range(4):
                ps = psProj.tile([P, SC], f32, name="ps_proj")
                for it in range(NT):
                    nc.tensor.matmul(
                        ps[:], valT_sb[:, it, st * P:(st + 1) * P],
                        wcv_sb[:, it, :],
                        start=(it == 0), stop=(it == NT - 1))
                cvt = str3.tile([P, L], f32, name="cvt")
                nc.vector.tensor_add(cvt[:], ps[:], bcv_sb[:])
                nc.sync.dma_start(cv_out[st * P:(st + 1) * P, :], cvt[:])

            # ---- cache_k = k2 @ W_DK.T + b_DK ----
            wdk_sb = wpool.tile([P, 16, L], f16, name="wdk_sb", tag="wbig")
            nc.sync.dma_start(wdk_sb[:], wdk)
            for st in range(4):
                ps = psProj.tile([P, SC], f32, name="ps_proj")
                for jt in range(16):
                    nc.tensor.matmul(
                        ps[:], k2T_sb[:, jt, st * P:(st + 1) * P],
                        wdk_sb[:, jt, :],
                        start=(jt == 0), stop=(jt == 15))
                ckt = str3.tile([P, L], f32, name="ckt")
                nc.vector.tensor_add(ckt[:], ps[:], bdk_sb[:])
                nc.sync.dma_start(ck_out[st * P:(st + 1) * P, :], ckt[:])

            # ---- phase Q: q2T = [q ; rope(qr_lin)] ----
            qryT_sb = load_xT(qryT, "qryT_sb")
            q2T_sb = acts.tile([P, 16, SC], f16, name="q2T_sb", tag="kq2T")
            outT128 = acts.tile([P, NT, SC], f16, name="outT128", tag="m8k")
            projT(q2T_sb, 0, wq, "wq", qryT_sb, bq_sb, rope=False)
            projT(q2T_sb, NT, wqr, "wqr", qryT_sb, bqr_sb, rope=True)

            # ---- gathered v ----
            for kt in range(16):
                nc.sync.dma_start(
                    v2_sb[:, kt, :],
                    cc_v_out[kt // 4, (kt % 4) * P:(kt % 4 + 1) * P, :])

            # ---- attention per head ----
            for h in range(H):
                k2h = k2hp.tile([P, 4, SC], f16, name="k2h")
                for r in range(4):
                    nc.sync.dma_start(
                        k2h[:, r, :], cc_k_out[r, h * P:(h + 1) * P, :])
                expTn = expp.tile([P, 16, SC], f16, name="expTn")
                for qt in range(4):
                    zh0 = zp.tile([P, 1], f32, name="zh0")
                    zh1 = zp.tile([P, 1], f32, name="zh1")
                    exp_sb = str3.tile([P, S], f16, name="exp_sb", bufs=2)
                    for half in range(2):
                        ps = psB.tile([P, 1024], f32, name="ps_scores")
                        for j in range(2):
                            kc = half * 2 + j
                            nc.tensor.matmul(
                                ps[:, j * 512:(j + 1) * 512],
                                q2T_sb[:, h, qt * P:(qt + 1) * P],
                                k2h[:, kc, :],
                                start=True, stop=True)
                        nc.scalar.activation(
                            exp_sb[:, half * 1024:(half + 1) * 1024], ps[:],
                            AF.Exp, scale=0.125,
                            accum_out=(zh0 if half == 0 else zh1)[:])
                    z = zp.tile([P, 1], f32, name="z")
                    nc.vector.tensor_add(z[:], zh0[:], zh1[:])
                    zi = zp.tile([P, 1], f32, name="zi")
                    nc.vector.reciprocal(zi[:], z[:])
                    wsb = str3.tile([P, S], f16, name="wsb", bufs=2)
                    nc.vector.tensor_scalar_mul(wsb[:], exp_sb[:], zi[:])
                    nc.sync.dma_start(
                        w_out[h, qt * P:(qt + 1) * P, :], wsb[:])
                    nc.sync.dma_start_transpose(
                        expTn[:, :, qt * P:(qt + 1) * P], wsb[:])
                psv = psAV.tile([P, SC], f32, name="ps_av")
                for kt in range(16):
                    nc.tensor.matmul(
                        psv[0:64, :], v2_sb[:, kt, h * HD:(h + 1) * HD],
                        expTn[:, kt, :],
                        start=(kt == 0), stop=(kt == 15))
                nc.vector.tensor_copy(outT64[:, h, :], psv[0:64, :])
                nc.sync.dma_start(
                    outT128[(h % 2) * 64:(h % 2 + 1) * 64, h // 2, :],
                    outT64[:, h, :])

            # ---- output projection ----
            wo_sb = wpool.tile([P, NT, E], f16, name="w_sb", tag="wbig")
            nc.sync.dma_start(wo_sb[:], wo)
            for qt in range(4):
                for oc in range(2):
                    ps = psProj.tile([P, SC], f32, name="ps_proj")
                    for it in range(NT):
                        nc.tensor.matmul(
                            ps[:], outT128[:, it, qt * P:(qt + 1) * P],
                            wo_sb[:, it, oc * 512:(oc + 1) * 512],
                            start=(it == 0), stop=(it == NT - 1))
                    osb = str3.tile([P, 512], f32, name="osb")
                    nc.vector.tensor_add(osb[:], ps[:],
                                         bo_sb[:, oc * 512:(oc + 1) * 512])
                    nc.sync.dma_start(
                        o_out[qt * P:(qt + 1) * P, oc * 512:(oc + 1) * 512],
                        osb[:])

    nc.compile()
    return nc


def _prep_host(query, key, value, consist,
               W_Q, b_Q, W_K, b_K, W_V, b_V, W_O, b_O,
               W_DQ, b_DQ, W_DKV, b_DKV, W_DK, b_DK,
               W_QR, b_QR, W_KR, b_KR):
    """Host-side folding/transposition. Returns per-core input maps."""
    f8 = np.float64
    f16 = np.float16
    f32 = np.float32

    def tiles_T(W):
        # W [out, in] -> fp16 [128, in//128, out]  (W.T tile layout)
        Wt = np.ascontiguousarray(W.T.astype(f16))       # [in, out]
        i_dim, o_dim = Wt.shape
        return np.ascontiguousarray(
            Wt.reshape(i_dim // P, P, o_dim).transpose(1, 0, 2))

    # folds (float64 for exactness)
    Wq64, Wk64, Wv64 = W_Q.astype(f8), W_K.astype(f8), W_V.astype(f8)
    Wdq64, Wdkv64 = W_DQ.astype(f8), W_DKV.astype(f8)
    Wqr64, Wkr64 = W_QR.astype(f8), W_KR.astype(f8)
    W_qr_fold = Wqr64 @ Wdq64 @ Wq64                       # [E, E]
    b_qr_fold = Wqr64 @ (Wdq64 @ b_Q.astype(f8) + b_DQ.astype(f8)) \
        + b_QR.astype(f8)
    W_kr_fold = Wkr64 @ Wdkv64 @ Wk64
    b_kr_fold = Wkr64 @ (Wdkv64 @ b_K.astype(f8) + b_DKV.astype(f8)) \
        + b_KR.astype(f8)
    W_cv = Wdkv64 @ Wv64                                   # [L, E]
    b_cv = Wdkv64 @ b_V.astype(f8) + b_DKV.astype(f8)

    def pp(b):
        # [E] -> [128, E//128] per-partition bias layout
        b = np.asarray(b, f32)
        return np.ascontiguousarray(b.reshape(-1, P).T)

    def bc(b):
        # [N] -> [128, N] broadcast layout
        b = np.asarray(b, f32)
        return np.ascontiguousarray(np.broadcast_to(b[None, :], (P, b.size)))

    # rope tables (reference-faithful fp32 semantics)
    cst = np.float32(float(np.asarray(consist)))
    expo = np.arange(0, E, 2, dtype=f32)[:E // 2]
    with np.errstate(over="ignore"):
        freq = (f32(1.0) / (cst ** expo)).astype(f32)      # overflow -> 0
    ang = np.arange(S, dtype=f32)[:, None] * freq[None, :]  # [S, half]
    cos_t, sin_t = np.cos(ang), np.sin(ang)                # [S, half]
    # transposed tables for features 0..127 (j = f//2; j>=64 are identity)
    j_idx = np.arange(P) // 2
    cos0_full = np.ascontiguousarray(cos_t[:, j_idx].T.astype(f16))  # [128,S]
    sin0_full = np.ascontiguousarray(sin_t[:, j_idx].T.astype(f16))

    perm = np.zeros((P, P), f16)
    jj = np.arange(0, P, 2)
    perm[jj, jj + 1] = 1.0      # P.T[2j+1, 2j] = 1
    perm[jj + 1, jj] = -1.0     # P.T[2j, 2j+1] = -1

    common = {
        "wq": tiles_T(W_Q), "wqr": tiles_T(W_qr_fold),
        "wk": tiles_T(W_K), "wkr": tiles_T(W_kr_fold),
        "wv": tiles_T(W_V), "wcv": tiles_T(W_cv),
        "wdk": tiles_T(W_DK), "wo": tiles_T(W_O),
        "bq": pp(b_Q), "bqr": pp(b_qr_fold),
        "bk": pp(b_K), "bkr": pp(b_kr_fold),
        "bv": bc(b_V), "bcv": bc(b_cv), "bdk": bc(b_DK), "bo": bc(b_O),
        "perm": perm,
    }

    def xT_tiles(x2d):
        # [SC, E] fp32 -> [128, 8, SC] fp16 transposed tiles
        xt = np.ascontiguousarray(x2d.T.astype(f16))       # [E, SC]
        return np.ascontiguousarray(
            xt.reshape(NT, P, SC).transpose(1, 0, 2))

    in_maps = []
    for c in range(NCORES):
        b_i, r = c // 4, c % 4
        sl = slice(r * SC, (r + 1) * SC)
        m = dict(common)
        m["qryT"] = xT_tiles(query[b_i, sl])
        m["keyT"] = xT_tiles(key[b_i, sl])
        m["valT"] = xT_tiles(value[b_i, sl])
        m["cos0"] = np.ascontiguousarray(cos0_full[:, sl])
        m["sin0"] = np.ascontiguousarray(sin0_full[:, sl])
        in_maps.append(m)
    return in_maps


def kernel(**inputs):
    from concourse.bass_utils import run_bass_kernel_spmd

    inputs = {k: (np.asarray(v) if not isinstance(v, (int, float)) else v)
              for k, v in inputs.items()}
    if "nc" not in _cache:
        _cache["nc"] = _build_nc()
    nc = _cache["nc"]

    in_maps = _prep_host(**inputs)
    res = run_bass_kernel_spmd(nc, in_maps, core_ids=list(range(NCORES)))
    _cache["last_results"] = res

    out = np.zeros((B, S, E), np.float32)
    weight = np.zeros((B, H, S, S), np.float32)
    cache_k = np.zeros((B, S, L), np.float32)
    cache_v = np.zeros((B, S, L), np.float32)
    for c in range(NCORES):
        b_i, r = c // 4, c % 4
        sl = slice(r * SC, (r + 1) * SC)
        rc = res.results[c]
        out[b_i, sl] = rc["o_out"]
        weight[b_i, :, sl, :] = rc["w_out"].astype(np.float32)
        cache_k[b_i, sl] = rc["ck_out"]
        cache_v[b_i, sl] = rc["cv_out"]
    return out, weight, cache_k, cache_v
